# revision 27
# baseline (speedup 1.0000x reference)
"""CANN recurrent update on 8 Trainium2 NeuronCores.

Computes prcn-1 steps of:
    temp = J @ r ;  U = temp + Iext ;  buf2 = (0.2*U)^2
    recSum = sum(k*buf2) ;  r' = buf2 * (0.04/recSum) / 0.04
returning (U_last, recSum_last, r_final).

Distribution: rows of J are sharded across 8 cores (2048 rows each, kept in
natural row-major layout — no host transpose).  Each step's GEMV is computed
on the Vector engine: one scalar_tensor_tensor per (128-row tile, 2048-col
window) with accum_out producing 128 exact-fp32 row dots per instruction.
The r vector is carried UNNORMALIZED between steps (J@(b/s) == (J@b)/s), so
the per-step AllGather of the local b chunk plus the partial recSum is the
only cross-core dependency; the scalar normalization folds into the next
step's elementwise chain.  J streaming from HBM (128 MiB/core/step) is the
roofline; the r0==0 first step needs no GEMV, so 13 of 14 steps stream J.
"""

import numpy as np

R, C = 128, 128
N = R * C                      # 16384 neurons
NCORES = 8
LOCAL = N // NCORES            # 2048 rows per core
NT = LOCAL // 128              # 16 row tiles per core
L = 2048                       # contraction window length
NW = N // L                    # 8 windows
WP = 2                         # windows per J-slab DMA (2 MiB slabs, 16KB lines)
CC_PAD = 2064                  # 2048 b values + 1 partial sum + pad (32B aligned)
RES = 10                       # position-0 J slabs kept resident in SBUF

ALPHA = np.float32(1.0)
BETA = np.float32(1.0)
KCONST = np.float32(0.005)
P2 = np.float32(0.2)
P04 = np.float32(0.04)

_COMPILED = {}


def _build(nsteps):
    """Build + compile the 8-core NEFF for `nsteps` recurrence steps
    (step 0 uses r0 == 0, i.e. no GEMV)."""
    import concourse.bass as bass
    import concourse.bass as bass
    import concourse.bacc as bacc
    import concourse.mybir as mybir
    import concourse.tile as tile

    F32 = mybir.dt.float32
    ALU = mybir.AluOpType
    AX = mybir.AxisListType

    nc = bacc.Bacc("TRN2", target_bir_lowering=False, debug=False,
                   num_devices=NCORES)

    jm_d = nc.dram_tensor("jm", [LOCAL, N], F32, kind="ExternalInput")
    iext_d = nc.dram_tensor("iext", [128, NT], F32, kind="ExternalInput")
    ident_d = nc.dram_tensor("ident", [128, 128], F32, kind="ExternalInput")
    u_d = nc.dram_tensor("u_out", [128, NT], F32, kind="ExternalOutput")
    b_d = nc.dram_tensor("b_out", [128, NT], F32, kind="ExternalOutput")
    s_d = nc.dram_tensor("s_out", [1, 1], F32, kind="ExternalOutput")

    rg = [list(range(NCORES))]
    last = nsteps - 1

    with tile.TileContext(nc) as tc:
        with (
            tc.tile_pool(name="const", bufs=1) as const,
            tc.tile_pool(name="jslab", bufs=8) as jslab,
            tc.tile_pool(name="scr", bufs=2) as scr,
            tc.tile_pool(name="rwin", bufs=2) as rwinp,
            tc.tile_pool(name="rlin", bufs=1) as rlinp,
            tc.tile_pool(name="small", bufs=2) as small,
            tc.tile_pool(name="ps", bufs=2, space="PSUM") as ps,
            tc.tile_pool(name="ps_rw", bufs=3, space="PSUM") as ps_rw,
            tc.tile_pool(name="dram", bufs=2, space="DRAM") as dram,
        ):
            ident = const.tile([128, 128], F32)
            nc.sync.dma_start(ident[:], ident_d[:])
            iext = const.tile([128, NT], F32)
            nc.sync.dma_start(iext[:], iext_d[:])
            ones = const.tile([128, 1], F32)
            nc.gpsimd.memset(ones[:], 1.0)
            ones_row = const.tile([1, 128], F32)
            nc.gpsimd.memset(ones_row[:], 1.0)
            c1 = const.tile([128, NT], F32)           # 0.2 * Iext
            nc.vector.tensor_scalar_mul(c1[:], iext[:], float(P2))

            # dummy barrier collective: absorbs cross-core launch skew while
            # the prologue (resident loads, step-0 elementwise) proceeds, so
            # the first real AllGather doesn't stall ~50us
            warm_in = dram.tile([1, 8], F32, name="warm_in")
            warm_out = dram.tile([NCORES, 8], F32, name="warm_out",
                                 addr_space="Shared")
            nc.gpsimd.dma_start(warm_in[:], ident_d[0:1, 0:8])
            nc.gpsimd.collective_compute(
                "AllGather", mybir.AluOpType.bypass,
                replica_groups=rg, ins=[warm_in[:]], outs=[warm_out[:]])

            # resident J slabs: position-0 (own window) tiles 0..RES-1 are
            # loaded once and reused every step — cuts HBM traffic and
            # removes the DMA burst right after each step boundary
            res_slabs = []
            for tl in range(RES):
                rs = const.tile([128, L], F32, name=f"res{tl}")
                nc.sync.dma_start(rs[:], jm_d[tl * 128:(tl + 1) * 128, 0:L])
                res_slabs.append(rs)

            # per-core window rotation: position p = global window
            # (core_id + p) % NCORES
            pid = nc.scalar.partition_id()
            wsnap = [None]
            for p in range(1, NW):
                reg = nc.scalar.alloc_register()
                nc.scalar.reg_add(reg, pid, p)
                nc.scalar.reg_alu(reg, reg, NW - 1, ALU.bitwise_and)
                wsnap.append(nc.scalar.snap(reg, donate=True,
                                            min_val=0, max_val=NW - 1))

            cc_out_prev = None
            cc_in_prev = None
            for t in range(nsteps):
                if t == 0:
                    b = small.tile([128, NT], F32, name="b")
                    sb_p = small.tile([128, 1], F32, name="sb_p")
                    # b = (0.2*Iext)^2 ; per-partition partial sums
                    nc.vector.scalar_tensor_tensor(
                        out=b[:], in0=c1[:], scalar=1.0, in1=c1[:],
                        op0=ALU.mult, op1=ALU.mult, accum_out=sb_p[:])
                    u = None
                else:
                    # ---- GEMV: z[p, tl] = sum_j J[tl*128+p, j] * r[j] ----
                    # Window POSITIONS are rotated per core (host rotates the
                    # jm column blocks): position p holds global window
                    # (core_id + p) % 8.  Position 0 is this core's OWN
                    # window, whose r data is in the locally staged cc_in —
                    # its 16 stt ops run with no AllGather dependency,
                    # hiding the collective latency.
                    parts = small.tile([128, NT, NW], F32, name="parts")
                    for p in range(NW):
                        rw = rwinp.tile([128, L], F32, name="rw")
                        if p == 0:
                            # own window from locally staged cc_in; issue on
                            # the SP queue so it is not FIFO-blocked behind
                            # the AG-dependent rl reads on the ACT queue
                            nc.sync.dma_start(
                                rw[:],
                                cc_in_prev[0:1, 0:L].broadcast_to([128, L]))
                        else:
                            # replicate r window to all partitions with no
                            # HBM re-reads: K=1 matmul ones_row.T @ r_row
                            # -> PSUM, idle ScalarE copies PSUM -> SBUF
                            rl = rlinp.tile([1, L], F32, name="rl")
                            nc.scalar.dma_start(
                                rl[:],
                                cc_out_prev[bass.ds(wsnap[p], 1), 0:L])
                            for q in range(L // 512):
                                pr = ps_rw.tile([128, 512], F32, name="pr")
                                nc.tensor.matmul(
                                    pr[:], ones_row[:],
                                    rl[0:1, q * 512:(q + 1) * 512],
                                    start=True, stop=True)
                                nc.scalar.copy(
                                    rw[:, q * 512:(q + 1) * 512], pr[:])
                        for tl in range(NT):
                            if p == 0 and tl < RES:
                                src_ap = res_slabs[tl][:]
                            else:
                                slab = jslab.tile([128, L], F32, name="slab")
                                nc.sync.dma_start(
                                    slab[:],
                                    jm_d[tl * 128:(tl + 1) * 128,
                                         p * L:(p + 1) * L])
                                src_ap = slab[:]
                            scratch = scr.tile([128, L], F32, name="scratch")
                            nc.vector.scalar_tensor_tensor(
                                out=scratch[:],
                                in0=src_ap,
                                scalar=1.0, in1=rw[:],
                                op0=ALU.mult, op1=ALU.mult,
                                accum_out=parts[:, tl, p:p + 1])
                    z = small.tile([128, NT], F32, name="z")
                    for tl in range(NT):
                        nc.vector.tensor_reduce(
                            out=z[:, tl:tl + 1], in_=parts[:, tl, :],
                            axis=AX.X, op=ALU.add)
                    # ---- scale from previous step's recSum partials ----
                    # read the 8 partials replicated onto all 128 partitions
                    sv = small.tile([128, NCORES], F32, name="sv")
                    nc.scalar.dma_start(
                        sv[:],
                        cc_out_prev[:, L:L + 1].rearrange("a b -> b a")
                        .broadcast_to([128, NCORES]))
                    s_prev = small.tile([128, 1], F32, name="s_prev")
                    nc.vector.tensor_reduce(out=s_prev[:], in_=sv[:],
                                            axis=AX.X, op=ALU.add)
                    scale_bc = small.tile([128, 1], F32, name="scale_bc")
                    nc.vector.reciprocal(scale_bc[:], s_prev[:])
                    b = small.tile([128, NT], F32, name="b")
                    sb_p = small.tile([128, 1], F32, name="sb_p")
                    if t < last:
                        # t2 = 0.2*U = z*(0.2/s) + 0.2*Iext ;  b = t2^2
                        w02 = small.tile([128, 1], F32, name="w02")
                        nc.vector.tensor_scalar_mul(w02[:], scale_bc[:], float(P2))
                        t2 = small.tile([128, NT], F32, name="t2")
                        nc.vector.scalar_tensor_tensor(
                            out=t2[:], in0=z[:], scalar=w02[:], in1=c1[:],
                            op0=ALU.mult, op1=ALU.add)
                        nc.vector.scalar_tensor_tensor(
                            out=b[:], in0=t2[:], scalar=1.0, in1=t2[:],
                            op0=ALU.mult, op1=ALU.mult, accum_out=sb_p[:])
                        u = None
                    else:
                        # final step materializes U itself
                        u = small.tile([128, NT], F32, name="u")
                        nc.vector.scalar_tensor_tensor(
                            out=u[:], in0=z[:], scalar=scale_bc[:], in1=iext[:],
                            op0=ALU.mult, op1=ALU.add)
                        v = small.tile([128, NT], F32, name="v")
                        nc.vector.tensor_scalar_mul(v[:], u[:], float(P2))
                        nc.vector.scalar_tensor_tensor(
                            out=b[:], in0=v[:], scalar=1.0, in1=v[:],
                            op0=ALU.mult, op1=ALU.mult, accum_out=sb_p[:])

                # ---- recSum partial: k * sum over partitions of sb_p ----
                ps_s = ps.tile([1, 1], F32, name="ps_s")
                nc.tensor.matmul(ps_s[:], ones[:], sb_p[:], start=True, stop=True)
                sp = small.tile([1, 1], F32, name="sp")
                nc.vector.tensor_scalar_mul(sp[:], ps_s[:], float(KCONST))

                if t < last:
                    # ---- transpose b to neuron order and all-gather ----
                    ps_bt = ps.tile([NT, 128], F32, name="ps_bt")
                    nc.tensor.transpose(ps_bt[:], b[:], ident[:])
                    bT = small.tile([NT, 128], F32, name="bT")
                    nc.vector.tensor_copy(bT[:], ps_bt[:])
                    cc_in = dram.tile([1, CC_PAD], F32, name="cc_in")
                    nc.gpsimd.dma_start(cc_in[0:1, 0:LOCAL], bT[:])
                    nc.gpsimd.dma_start(cc_in[0:1, L:L + 1], sp[:])
                    cc_out = dram.tile([NCORES, CC_PAD], F32, name="cc_out",
                                       addr_space="Shared")
                    nc.gpsimd.collective_compute(
                        "AllGather", mybir.AluOpType.bypass,
                        replica_groups=rg, ins=[cc_in[:]], outs=[cc_out[:]])
                    cc_out_prev = cc_out
                    cc_in_prev = cc_in
                    cc_in_prev = cc_in
                else:
                    if u is None:       # nsteps == 1 corner: U = Iext
                        u = small.tile([128, NT], F32, name="u")
                        nc.vector.tensor_copy(u[:], iext[:])
                    nc.gpsimd.dma_start(u_d[:], u[:])
                    nc.gpsimd.dma_start(b_d[:], b[:])
                    nc.gpsimd.dma_start(s_d[:], sp[:])

    nc.compile()
    return nc


def _get_compiled(nsteps):
    if nsteps not in _COMPILED:
        _COMPILED[nsteps] = _build(nsteps)
    return _COMPILED[nsteps]


def _run(nc, in_maps, **kwargs):
    from concourse.bass_utils import run_bass_kernel_spmd
    return run_bass_kernel_spmd(nc, in_maps, core_ids=list(range(NCORES)),
                                **kwargs)


def _numpy_fallback(net_Iext, J, net_r0, prcn):
    """Reference-shaped fp32 numpy path (used only if r0 != 0)."""
    J32 = np.asarray(J, dtype=np.float32)
    I32 = np.asarray(net_Iext, dtype=np.float32).ravel()
    r = np.asarray(net_r0, dtype=np.float32).ravel()
    for _ in range(int(prcn) - 1):
        temp = (J32 @ r).astype(np.float32)
        U = (temp + I32).astype(np.float32)
        b = ((P2 * U).astype(np.float32)) ** 2
        s = (KCONST * b).sum(dtype=np.float32)
        r = (b * (P04 / s) / P04).astype(np.float32)
    return (U.reshape(R, C), np.float32(s), r.reshape(R, C))


def kernel(net_Iext, J, net_r0, prcn, _trace=False, _nc_cache=None):
    net_Iext = np.ascontiguousarray(np.asarray(net_Iext, dtype=np.float32))
    J = np.ascontiguousarray(np.asarray(J, dtype=np.float32))
    net_r0 = np.asarray(net_r0, dtype=np.float32)
    prcn = int(prcn)
    nsteps = prcn - 1
    assert nsteps >= 1, "prcn must be >= 2"
    assert net_Iext.shape == (R, C) and J.shape == (N, N)

    if np.any(net_r0 != 0):
        return _numpy_fallback(net_Iext, J, net_r0, prcn)

    nc = _nc_cache if _nc_cache is not None else _get_compiled(nsteps)

    ident = np.eye(128, dtype=np.float32)
    iflat = net_Iext.ravel()
    in_maps = []
    for m in range(NCORES):
        # rotate window blocks so loop position p holds global window
        # (m + p) % NW  (position 0 = this core's own window)
        rows = J[m * LOCAL:(m + 1) * LOCAL, :]
        jm = np.empty_like(rows)
        for p in range(NW):
            w = (m + p) % NW
            jm[:, p * L:(p + 1) * L] = rows[:, w * L:(w + 1) * L]
        in_maps.append({
            "jm": jm,
            "iext": np.ascontiguousarray(
                iflat[m * LOCAL:(m + 1) * LOCAL].reshape(NT, 128).T),
            "ident": ident,
        })

    res = _run(nc, in_maps, trace=_trace)

    u_full = np.empty(N, dtype=np.float32)
    b_full = np.empty(N, dtype=np.float32)
    s_parts = []
    for m in range(NCORES):
        r_m = res.results[m]
        u_full[m * LOCAL:(m + 1) * LOCAL] = r_m["u_out"].T.ravel()
        b_full[m * LOCAL:(m + 1) * LOCAL] = r_m["b_out"].T.ravel()
        s_parts.append(np.float32(r_m["s_out"][0, 0]))
    s = np.float32(0.0)
    for p in s_parts:
        s = np.float32(s + p)
    r_full = ((b_full * (P04 / s)) / P04).astype(np.float32)

    out = (u_full.reshape(R, C), s, r_full.reshape(R, C))
    if _trace:
        out = (out, res)
    return out


# revision 28
# speedup vs baseline: 1.0066x; 1.0066x over previous
"""CANN recurrent update on 8 Trainium2 NeuronCores.

Computes prcn-1 steps of:
    temp = J @ r ;  U = temp + Iext ;  buf2 = (0.2*U)^2
    recSum = sum(k*buf2) ;  r' = buf2 * (0.04/recSum) / 0.04
returning (U_last, recSum_last, r_final).

Distribution: rows of J are sharded across 8 cores (2048 rows each, kept in
natural row-major layout — no host transpose).  Each step's GEMV is computed
on the Vector engine: one scalar_tensor_tensor per (128-row tile, 2048-col
window) with accum_out producing 128 exact-fp32 row dots per instruction.
The r vector is carried UNNORMALIZED between steps (J@(b/s) == (J@b)/s), so
the per-step AllGather of the local b chunk plus the partial recSum is the
only cross-core dependency; the scalar normalization folds into the next
step's elementwise chain.  J streaming from HBM (128 MiB/core/step) is the
roofline; the r0==0 first step needs no GEMV, so 13 of 14 steps stream J.
"""

import numpy as np

R, C = 128, 128
N = R * C                      # 16384 neurons
NCORES = 8
LOCAL = N // NCORES            # 2048 rows per core
NT = LOCAL // 128              # 16 row tiles per core
L = 2048                       # contraction window length
NW = N // L                    # 8 windows
WP = 2                         # windows per J-slab DMA (2 MiB slabs, 16KB lines)
CC_PAD = 2064                  # 2048 b values + 1 partial sum + pad (32B aligned)
RES = 10                       # position-0 J slabs kept resident in SBUF

ALPHA = np.float32(1.0)
BETA = np.float32(1.0)
KCONST = np.float32(0.005)
P2 = np.float32(0.2)
P04 = np.float32(0.04)

_COMPILED = {}


def _build(nsteps):
    """Build + compile the 8-core NEFF for `nsteps` recurrence steps
    (step 0 uses r0 == 0, i.e. no GEMV)."""
    import concourse.bass as bass
    import concourse.bass as bass
    import concourse.bacc as bacc
    import concourse.mybir as mybir
    import concourse.tile as tile

    F32 = mybir.dt.float32
    ALU = mybir.AluOpType
    AX = mybir.AxisListType

    nc = bacc.Bacc("TRN2", target_bir_lowering=False, debug=False,
                   num_devices=NCORES)

    jm_d = nc.dram_tensor("jm", [LOCAL, N], F32, kind="ExternalInput")
    iext_d = nc.dram_tensor("iext", [128, NT], F32, kind="ExternalInput")
    ident_d = nc.dram_tensor("ident", [128, 128], F32, kind="ExternalInput")
    u_d = nc.dram_tensor("u_out", [128, NT], F32, kind="ExternalOutput")
    b_d = nc.dram_tensor("b_out", [128, NT], F32, kind="ExternalOutput")
    s_d = nc.dram_tensor("s_out", [1, 1], F32, kind="ExternalOutput")

    rg = [list(range(NCORES))]
    last = nsteps - 1

    with tile.TileContext(nc) as tc:
        with (
            tc.tile_pool(name="const", bufs=1) as const,
            tc.tile_pool(name="jslab", bufs=8) as jslab,
            tc.tile_pool(name="scr", bufs=2) as scr,
            tc.tile_pool(name="rwin", bufs=2) as rwinp,
            tc.tile_pool(name="rlin", bufs=1) as rlinp,
            tc.tile_pool(name="small", bufs=2) as small,
            tc.tile_pool(name="ps", bufs=2, space="PSUM") as ps,
            tc.tile_pool(name="ps_rw", bufs=3, space="PSUM") as ps_rw,
            tc.tile_pool(name="dram", bufs=2, space="DRAM") as dram,
        ):
            ident = const.tile([128, 128], F32)
            nc.sync.dma_start(ident[:], ident_d[:])
            iext = const.tile([128, NT], F32)
            nc.sync.dma_start(iext[:], iext_d[:])
            ones = const.tile([128, 1], F32)
            nc.gpsimd.memset(ones[:], 1.0)
            ones_row = const.tile([1, 128], F32)
            nc.gpsimd.memset(ones_row[:], 1.0)
            c1 = const.tile([128, NT], F32)           # 0.2 * Iext
            nc.vector.tensor_scalar_mul(c1[:], iext[:], float(P2))

            # resident J slabs: position-0 (own window) tiles 0..RES-1 are
            # loaded once and reused every step — cuts HBM traffic and
            # removes the DMA burst right after each step boundary
            res_slabs = []
            for tl in range(RES):
                rs = const.tile([128, L], F32, name=f"res{tl}")
                nc.sync.dma_start(rs[:], jm_d[tl * 128:(tl + 1) * 128, 0:L])
                res_slabs.append(rs)

            # per-core window rotation: position p = global window
            # (core_id + p) % NCORES
            pid = nc.scalar.partition_id()
            wsnap = [None]
            for p in range(1, NW):
                reg = nc.scalar.alloc_register()
                nc.scalar.reg_add(reg, pid, p)
                nc.scalar.reg_alu(reg, reg, NW - 1, ALU.bitwise_and)
                wsnap.append(nc.scalar.snap(reg, donate=True,
                                            min_val=0, max_val=NW - 1))

            cc_out_prev = None
            cc_in_prev = None
            for t in range(nsteps):
                if t == 0:
                    b = small.tile([128, NT], F32, name="b")
                    sb_p = small.tile([128, 1], F32, name="sb_p")
                    # b = (0.2*Iext)^2 ; per-partition partial sums
                    nc.vector.scalar_tensor_tensor(
                        out=b[:], in0=c1[:], scalar=1.0, in1=c1[:],
                        op0=ALU.mult, op1=ALU.mult, accum_out=sb_p[:])
                    u = None
                else:
                    # ---- GEMV: z[p, tl] = sum_j J[tl*128+p, j] * r[j] ----
                    # Window POSITIONS are rotated per core (host rotates the
                    # jm column blocks): position p holds global window
                    # (core_id + p) % 8.  Position 0 is this core's OWN
                    # window, whose r data is in the locally staged cc_in —
                    # its 16 stt ops run with no AllGather dependency,
                    # hiding the collective latency.
                    parts = small.tile([128, NT, NW], F32, name="parts")
                    for p in range(NW):
                        rw = rwinp.tile([128, L], F32, name="rw")
                        if p == 0:
                            # own window from locally staged cc_in; issue on
                            # the SP queue so it is not FIFO-blocked behind
                            # the AG-dependent rl reads on the ACT queue
                            nc.sync.dma_start(
                                rw[:],
                                cc_in_prev[0:1, 0:L].broadcast_to([128, L]))
                        else:
                            # replicate r window to all partitions with no
                            # HBM re-reads: K=1 matmul ones_row.T @ r_row
                            # -> PSUM, idle ScalarE copies PSUM -> SBUF
                            rl = rlinp.tile([1, L], F32, name="rl")
                            nc.scalar.dma_start(
                                rl[:],
                                cc_out_prev[bass.ds(wsnap[p], 1), 0:L])
                            for q in range(L // 512):
                                pr = ps_rw.tile([128, 512], F32, name="pr")
                                nc.tensor.matmul(
                                    pr[:], ones_row[:],
                                    rl[0:1, q * 512:(q + 1) * 512],
                                    start=True, stop=True)
                                nc.scalar.copy(
                                    rw[:, q * 512:(q + 1) * 512], pr[:])
                        for tl in range(NT):
                            if p == 0 and tl < RES:
                                src_ap = res_slabs[tl][:]
                            else:
                                slab = jslab.tile([128, L], F32, name="slab")
                                nc.sync.dma_start(
                                    slab[:],
                                    jm_d[tl * 128:(tl + 1) * 128,
                                         p * L:(p + 1) * L])
                                src_ap = slab[:]
                            scratch = scr.tile([128, L], F32, name="scratch")
                            nc.vector.scalar_tensor_tensor(
                                out=scratch[:],
                                in0=src_ap,
                                scalar=1.0, in1=rw[:],
                                op0=ALU.mult, op1=ALU.mult,
                                accum_out=parts[:, tl, p:p + 1])
                    z = small.tile([128, NT], F32, name="z")
                    for tl in range(NT):
                        nc.vector.tensor_reduce(
                            out=z[:, tl:tl + 1], in_=parts[:, tl, :],
                            axis=AX.X, op=ALU.add)
                    # ---- scale from previous step's recSum partials ----
                    # read the 8 partials replicated onto all 128 partitions
                    sv = small.tile([128, NCORES], F32, name="sv")
                    nc.scalar.dma_start(
                        sv[:],
                        cc_out_prev[:, L:L + 1].rearrange("a b -> b a")
                        .broadcast_to([128, NCORES]))
                    s_prev = small.tile([128, 1], F32, name="s_prev")
                    nc.vector.tensor_reduce(out=s_prev[:], in_=sv[:],
                                            axis=AX.X, op=ALU.add)
                    scale_bc = small.tile([128, 1], F32, name="scale_bc")
                    nc.vector.reciprocal(scale_bc[:], s_prev[:])
                    b = small.tile([128, NT], F32, name="b")
                    sb_p = small.tile([128, 1], F32, name="sb_p")
                    if t < last:
                        # t2 = 0.2*U = z*(0.2/s) + 0.2*Iext ;  b = t2^2
                        w02 = small.tile([128, 1], F32, name="w02")
                        nc.vector.tensor_scalar_mul(w02[:], scale_bc[:], float(P2))
                        t2 = small.tile([128, NT], F32, name="t2")
                        nc.vector.scalar_tensor_tensor(
                            out=t2[:], in0=z[:], scalar=w02[:], in1=c1[:],
                            op0=ALU.mult, op1=ALU.add)
                        nc.vector.scalar_tensor_tensor(
                            out=b[:], in0=t2[:], scalar=1.0, in1=t2[:],
                            op0=ALU.mult, op1=ALU.mult, accum_out=sb_p[:])
                        u = None
                    else:
                        # final step materializes U itself
                        u = small.tile([128, NT], F32, name="u")
                        nc.vector.scalar_tensor_tensor(
                            out=u[:], in0=z[:], scalar=scale_bc[:], in1=iext[:],
                            op0=ALU.mult, op1=ALU.add)
                        v = small.tile([128, NT], F32, name="v")
                        nc.vector.tensor_scalar_mul(v[:], u[:], float(P2))
                        nc.vector.scalar_tensor_tensor(
                            out=b[:], in0=v[:], scalar=1.0, in1=v[:],
                            op0=ALU.mult, op1=ALU.mult, accum_out=sb_p[:])

                # ---- recSum partial: k * sum over partitions of sb_p ----
                ps_s = ps.tile([1, 1], F32, name="ps_s")
                nc.tensor.matmul(ps_s[:], ones[:], sb_p[:], start=True, stop=True)
                sp = small.tile([1, 1], F32, name="sp")
                nc.vector.tensor_scalar_mul(sp[:], ps_s[:], float(KCONST))

                if t < last:
                    # ---- transpose b to neuron order and all-gather ----
                    ps_bt = ps.tile([NT, 128], F32, name="ps_bt")
                    nc.tensor.transpose(ps_bt[:], b[:], ident[:])
                    bT = small.tile([NT, 128], F32, name="bT")
                    nc.vector.tensor_copy(bT[:], ps_bt[:])
                    cc_in = dram.tile([1, CC_PAD], F32, name="cc_in")
                    nc.gpsimd.dma_start(cc_in[0:1, 0:LOCAL], bT[:])
                    nc.gpsimd.dma_start(cc_in[0:1, L:L + 1], sp[:])
                    cc_out = dram.tile([NCORES, CC_PAD], F32, name="cc_out",
                                       addr_space="Shared")
                    nc.gpsimd.collective_compute(
                        "AllGather", mybir.AluOpType.bypass,
                        replica_groups=rg, ins=[cc_in[:]], outs=[cc_out[:]])
                    cc_out_prev = cc_out
                    cc_in_prev = cc_in
                    cc_in_prev = cc_in
                else:
                    if u is None:       # nsteps == 1 corner: U = Iext
                        u = small.tile([128, NT], F32, name="u")
                        nc.vector.tensor_copy(u[:], iext[:])
                    nc.gpsimd.dma_start(u_d[:], u[:])
                    nc.gpsimd.dma_start(b_d[:], b[:])
                    nc.gpsimd.dma_start(s_d[:], sp[:])

    nc.compile()
    return nc


def _get_compiled(nsteps):
    if nsteps not in _COMPILED:
        _COMPILED[nsteps] = _build(nsteps)
    return _COMPILED[nsteps]


def _run(nc, in_maps, **kwargs):
    from concourse.bass_utils import run_bass_kernel_spmd
    return run_bass_kernel_spmd(nc, in_maps, core_ids=list(range(NCORES)),
                                **kwargs)


def _numpy_fallback(net_Iext, J, net_r0, prcn):
    """Reference-shaped fp32 numpy path (used only if r0 != 0)."""
    J32 = np.asarray(J, dtype=np.float32)
    I32 = np.asarray(net_Iext, dtype=np.float32).ravel()
    r = np.asarray(net_r0, dtype=np.float32).ravel()
    for _ in range(int(prcn) - 1):
        temp = (J32 @ r).astype(np.float32)
        U = (temp + I32).astype(np.float32)
        b = ((P2 * U).astype(np.float32)) ** 2
        s = (KCONST * b).sum(dtype=np.float32)
        r = (b * (P04 / s) / P04).astype(np.float32)
    return (U.reshape(R, C), np.float32(s), r.reshape(R, C))


def kernel(net_Iext, J, net_r0, prcn, _trace=False, _nc_cache=None):
    net_Iext = np.ascontiguousarray(np.asarray(net_Iext, dtype=np.float32))
    J = np.ascontiguousarray(np.asarray(J, dtype=np.float32))
    net_r0 = np.asarray(net_r0, dtype=np.float32)
    prcn = int(prcn)
    nsteps = prcn - 1
    assert nsteps >= 1, "prcn must be >= 2"
    assert net_Iext.shape == (R, C) and J.shape == (N, N)

    if np.any(net_r0 != 0):
        return _numpy_fallback(net_Iext, J, net_r0, prcn)

    nc = _nc_cache if _nc_cache is not None else _get_compiled(nsteps)

    ident = np.eye(128, dtype=np.float32)
    iflat = net_Iext.ravel()
    in_maps = []
    for m in range(NCORES):
        # rotate window blocks so loop position p holds global window
        # (m + p) % NW  (position 0 = this core's own window)
        rows = J[m * LOCAL:(m + 1) * LOCAL, :]
        jm = np.empty_like(rows)
        for p in range(NW):
            w = (m + p) % NW
            jm[:, p * L:(p + 1) * L] = rows[:, w * L:(w + 1) * L]
        in_maps.append({
            "jm": jm,
            "iext": np.ascontiguousarray(
                iflat[m * LOCAL:(m + 1) * LOCAL].reshape(NT, 128).T),
            "ident": ident,
        })

    res = _run(nc, in_maps, trace=_trace)

    u_full = np.empty(N, dtype=np.float32)
    b_full = np.empty(N, dtype=np.float32)
    s_parts = []
    for m in range(NCORES):
        r_m = res.results[m]
        u_full[m * LOCAL:(m + 1) * LOCAL] = r_m["u_out"].T.ravel()
        b_full[m * LOCAL:(m + 1) * LOCAL] = r_m["b_out"].T.ravel()
        s_parts.append(np.float32(r_m["s_out"][0, 0]))
    s = np.float32(0.0)
    for p in s_parts:
        s = np.float32(s + p)
    r_full = ((b_full * (P04 / s)) / P04).astype(np.float32)

    out = (u_full.reshape(R, C), s, r_full.reshape(R, C))
    if _trace:
        out = (out, res)
    return out


# revision 29
# speedup vs baseline: 1.0102x; 1.0036x over previous
"""CANN recurrent update on 8 Trainium2 NeuronCores.

Computes prcn-1 steps of:
    temp = J @ r ;  U = temp + Iext ;  buf2 = (0.2*U)^2
    recSum = sum(k*buf2) ;  r' = buf2 * (0.04/recSum) / 0.04
returning (U_last, recSum_last, r_final).

Distribution: rows of J are sharded across 8 cores (2048 rows each, kept in
natural row-major layout — no host transpose).  Each step's GEMV is computed
on the Vector engine: one scalar_tensor_tensor per (128-row tile, 2048-col
window) with accum_out producing 128 exact-fp32 row dots per instruction.
The r vector is carried UNNORMALIZED between steps (J@(b/s) == (J@b)/s), so
the per-step AllGather of the local b chunk plus the partial recSum is the
only cross-core dependency; the scalar normalization folds into the next
step's elementwise chain.  J streaming from HBM (128 MiB/core/step) is the
roofline; the r0==0 first step needs no GEMV, so 13 of 14 steps stream J.
"""

import numpy as np

R, C = 128, 128
N = R * C                      # 16384 neurons
NCORES = 8
LOCAL = N // NCORES            # 2048 rows per core
NT = LOCAL // 128              # 16 row tiles per core
L = 2048                       # contraction window length
NW = N // L                    # 8 windows
WP = 2                         # windows per J-slab DMA (2 MiB slabs, 16KB lines)
CC_PAD = 2064                  # 2048 b values + 1 partial sum + pad (32B aligned)
RES = 11                       # position-0 J slabs kept resident in SBUF

ALPHA = np.float32(1.0)
BETA = np.float32(1.0)
KCONST = np.float32(0.005)
P2 = np.float32(0.2)
P04 = np.float32(0.04)

_COMPILED = {}


def _build(nsteps):
    """Build + compile the 8-core NEFF for `nsteps` recurrence steps
    (step 0 uses r0 == 0, i.e. no GEMV)."""
    import concourse.bass as bass
    import concourse.bass as bass
    import concourse.bacc as bacc
    import concourse.mybir as mybir
    import concourse.tile as tile

    F32 = mybir.dt.float32
    ALU = mybir.AluOpType
    AX = mybir.AxisListType

    nc = bacc.Bacc("TRN2", target_bir_lowering=False, debug=False,
                   num_devices=NCORES)

    jm_d = nc.dram_tensor("jm", [LOCAL, N], F32, kind="ExternalInput")
    iext_d = nc.dram_tensor("iext", [128, NT], F32, kind="ExternalInput")
    ident_d = nc.dram_tensor("ident", [128, 128], F32, kind="ExternalInput")
    u_d = nc.dram_tensor("u_out", [128, NT], F32, kind="ExternalOutput")
    b_d = nc.dram_tensor("b_out", [128, NT], F32, kind="ExternalOutput")
    s_d = nc.dram_tensor("s_out", [1, 1], F32, kind="ExternalOutput")

    rg = [list(range(NCORES))]
    last = nsteps - 1

    with tile.TileContext(nc) as tc:
        with (
            tc.tile_pool(name="const", bufs=1) as const,
            tc.tile_pool(name="jslab", bufs=8) as jslab,
            tc.tile_pool(name="scr", bufs=1) as scr,
            tc.tile_pool(name="rwin", bufs=2) as rwinp,
            tc.tile_pool(name="rlin", bufs=1) as rlinp,
            tc.tile_pool(name="small", bufs=2) as small,
            tc.tile_pool(name="ps", bufs=2, space="PSUM") as ps,
            tc.tile_pool(name="ps_rw", bufs=3, space="PSUM") as ps_rw,
            tc.tile_pool(name="dram", bufs=2, space="DRAM") as dram,
        ):
            ident = const.tile([128, 128], F32)
            nc.sync.dma_start(ident[:], ident_d[:])
            iext = const.tile([128, NT], F32)
            nc.sync.dma_start(iext[:], iext_d[:])
            ones = const.tile([128, 1], F32)
            nc.gpsimd.memset(ones[:], 1.0)
            ones_row = const.tile([1, 128], F32)
            nc.gpsimd.memset(ones_row[:], 1.0)
            c1 = const.tile([128, NT], F32)           # 0.2 * Iext
            nc.vector.tensor_scalar_mul(c1[:], iext[:], float(P2))

            # resident J slabs: position-0 (own window) tiles 0..RES-1 are
            # loaded once and reused every step — cuts HBM traffic and
            # removes the DMA burst right after each step boundary
            res_slabs = []
            for tl in range(RES):
                rs = const.tile([128, L], F32, name=f"res{tl}")
                nc.sync.dma_start(rs[:], jm_d[tl * 128:(tl + 1) * 128, 0:L])
                res_slabs.append(rs)

            # per-core window rotation: position p = global window
            # (core_id + p) % NCORES
            pid = nc.scalar.partition_id()
            wsnap = [None]
            for p in range(1, NW):
                reg = nc.scalar.alloc_register()
                nc.scalar.reg_add(reg, pid, p)
                nc.scalar.reg_alu(reg, reg, NW - 1, ALU.bitwise_and)
                wsnap.append(nc.scalar.snap(reg, donate=True,
                                            min_val=0, max_val=NW - 1))

            cc_out_prev = None
            cc_in_prev = None
            for t in range(nsteps):
                if t == 0:
                    b = small.tile([128, NT], F32, name="b")
                    sb_p = small.tile([128, 1], F32, name="sb_p")
                    # b = (0.2*Iext)^2 ; per-partition partial sums
                    nc.vector.scalar_tensor_tensor(
                        out=b[:], in0=c1[:], scalar=1.0, in1=c1[:],
                        op0=ALU.mult, op1=ALU.mult, accum_out=sb_p[:])
                    u = None
                else:
                    # ---- GEMV: z[p, tl] = sum_j J[tl*128+p, j] * r[j] ----
                    # Window POSITIONS are rotated per core (host rotates the
                    # jm column blocks): position p holds global window
                    # (core_id + p) % 8.  Position 0 is this core's OWN
                    # window, whose r data is in the locally staged cc_in —
                    # its 16 stt ops run with no AllGather dependency,
                    # hiding the collective latency.
                    parts = small.tile([128, NT, NW], F32, name="parts")
                    for p in range(NW):
                        rw = rwinp.tile([128, L], F32, name="rw")
                        if p == 0:
                            # own window from locally staged cc_in; issue on
                            # the SP queue so it is not FIFO-blocked behind
                            # the AG-dependent rl reads on the ACT queue
                            nc.sync.dma_start(
                                rw[:],
                                cc_in_prev[0:1, 0:L].broadcast_to([128, L]))
                        else:
                            # replicate r window to all partitions with no
                            # HBM re-reads: K=1 matmul ones_row.T @ r_row
                            # -> PSUM, idle ScalarE copies PSUM -> SBUF
                            rl = rlinp.tile([1, L], F32, name="rl")
                            nc.scalar.dma_start(
                                rl[:],
                                cc_out_prev[bass.ds(wsnap[p], 1), 0:L])
                            for q in range(L // 512):
                                pr = ps_rw.tile([128, 512], F32, name="pr")
                                nc.tensor.matmul(
                                    pr[:], ones_row[:],
                                    rl[0:1, q * 512:(q + 1) * 512],
                                    start=True, stop=True)
                                nc.scalar.copy(
                                    rw[:, q * 512:(q + 1) * 512], pr[:])
                        for tl in range(NT):
                            if p == 0 and tl < RES:
                                # resident slab must survive -> scratch out
                                src_ap = res_slabs[tl][:]
                                out_ap = scr.tile([128, L], F32,
                                                  name="scratch")[:]
                            else:
                                # streamed slab is dead after this op:
                                # compute in place
                                slab = jslab.tile([128, L], F32, name="slab")
                                nc.sync.dma_start(
                                    slab[:],
                                    jm_d[tl * 128:(tl + 1) * 128,
                                         p * L:(p + 1) * L])
                                src_ap = slab[:]
                                out_ap = slab[:]
                            nc.vector.scalar_tensor_tensor(
                                out=out_ap,
                                in0=src_ap,
                                scalar=1.0, in1=rw[:],
                                op0=ALU.mult, op1=ALU.mult,
                                accum_out=parts[:, tl, p:p + 1])
                    z = small.tile([128, NT], F32, name="z")
                    for tl in range(NT):
                        nc.vector.tensor_reduce(
                            out=z[:, tl:tl + 1], in_=parts[:, tl, :],
                            axis=AX.X, op=ALU.add)
                    # ---- scale from previous step's recSum partials ----
                    # read the 8 partials replicated onto all 128 partitions
                    sv = small.tile([128, NCORES], F32, name="sv")
                    nc.scalar.dma_start(
                        sv[:],
                        cc_out_prev[:, L:L + 1].rearrange("a b -> b a")
                        .broadcast_to([128, NCORES]))
                    s_prev = small.tile([128, 1], F32, name="s_prev")
                    nc.vector.tensor_reduce(out=s_prev[:], in_=sv[:],
                                            axis=AX.X, op=ALU.add)
                    scale_bc = small.tile([128, 1], F32, name="scale_bc")
                    nc.vector.reciprocal(scale_bc[:], s_prev[:])
                    b = small.tile([128, NT], F32, name="b")
                    sb_p = small.tile([128, 1], F32, name="sb_p")
                    if t < last:
                        # t2 = 0.2*U = z*(0.2/s) + 0.2*Iext ;  b = t2^2
                        w02 = small.tile([128, 1], F32, name="w02")
                        nc.vector.tensor_scalar_mul(w02[:], scale_bc[:], float(P2))
                        t2 = small.tile([128, NT], F32, name="t2")
                        nc.vector.scalar_tensor_tensor(
                            out=t2[:], in0=z[:], scalar=w02[:], in1=c1[:],
                            op0=ALU.mult, op1=ALU.add)
                        nc.vector.scalar_tensor_tensor(
                            out=b[:], in0=t2[:], scalar=1.0, in1=t2[:],
                            op0=ALU.mult, op1=ALU.mult, accum_out=sb_p[:])
                        u = None
                    else:
                        # final step materializes U itself
                        u = small.tile([128, NT], F32, name="u")
                        nc.vector.scalar_tensor_tensor(
                            out=u[:], in0=z[:], scalar=scale_bc[:], in1=iext[:],
                            op0=ALU.mult, op1=ALU.add)
                        v = small.tile([128, NT], F32, name="v")
                        nc.vector.tensor_scalar_mul(v[:], u[:], float(P2))
                        nc.vector.scalar_tensor_tensor(
                            out=b[:], in0=v[:], scalar=1.0, in1=v[:],
                            op0=ALU.mult, op1=ALU.mult, accum_out=sb_p[:])

                # ---- recSum partial: k * sum over partitions of sb_p ----
                ps_s = ps.tile([1, 1], F32, name="ps_s")
                nc.tensor.matmul(ps_s[:], ones[:], sb_p[:], start=True, stop=True)
                sp = small.tile([1, 1], F32, name="sp")
                nc.vector.tensor_scalar_mul(sp[:], ps_s[:], float(KCONST))

                if t < last:
                    # ---- transpose b to neuron order and all-gather ----
                    ps_bt = ps.tile([NT, 128], F32, name="ps_bt")
                    nc.tensor.transpose(ps_bt[:], b[:], ident[:])
                    bT = small.tile([NT, 128], F32, name="bT")
                    nc.vector.tensor_copy(bT[:], ps_bt[:])
                    cc_in = dram.tile([1, CC_PAD], F32, name="cc_in")
                    nc.gpsimd.dma_start(cc_in[0:1, 0:LOCAL], bT[:])
                    nc.gpsimd.dma_start(cc_in[0:1, L:L + 1], sp[:])
                    cc_out = dram.tile([NCORES, CC_PAD], F32, name="cc_out",
                                       addr_space="Shared")
                    nc.gpsimd.collective_compute(
                        "AllGather", mybir.AluOpType.bypass,
                        replica_groups=rg, ins=[cc_in[:]], outs=[cc_out[:]])
                    cc_out_prev = cc_out
                    cc_in_prev = cc_in
                    cc_in_prev = cc_in
                else:
                    if u is None:       # nsteps == 1 corner: U = Iext
                        u = small.tile([128, NT], F32, name="u")
                        nc.vector.tensor_copy(u[:], iext[:])
                    nc.gpsimd.dma_start(u_d[:], u[:])
                    nc.gpsimd.dma_start(b_d[:], b[:])
                    nc.gpsimd.dma_start(s_d[:], sp[:])

    nc.compile()
    return nc


def _get_compiled(nsteps):
    if nsteps not in _COMPILED:
        _COMPILED[nsteps] = _build(nsteps)
    return _COMPILED[nsteps]


def _run(nc, in_maps, **kwargs):
    from concourse.bass_utils import run_bass_kernel_spmd
    return run_bass_kernel_spmd(nc, in_maps, core_ids=list(range(NCORES)),
                                **kwargs)


def _numpy_fallback(net_Iext, J, net_r0, prcn):
    """Reference-shaped fp32 numpy path (used only if r0 != 0)."""
    J32 = np.asarray(J, dtype=np.float32)
    I32 = np.asarray(net_Iext, dtype=np.float32).ravel()
    r = np.asarray(net_r0, dtype=np.float32).ravel()
    for _ in range(int(prcn) - 1):
        temp = (J32 @ r).astype(np.float32)
        U = (temp + I32).astype(np.float32)
        b = ((P2 * U).astype(np.float32)) ** 2
        s = (KCONST * b).sum(dtype=np.float32)
        r = (b * (P04 / s) / P04).astype(np.float32)
    return (U.reshape(R, C), np.float32(s), r.reshape(R, C))


def kernel(net_Iext, J, net_r0, prcn, _trace=False, _nc_cache=None):
    net_Iext = np.ascontiguousarray(np.asarray(net_Iext, dtype=np.float32))
    J = np.ascontiguousarray(np.asarray(J, dtype=np.float32))
    net_r0 = np.asarray(net_r0, dtype=np.float32)
    prcn = int(prcn)
    nsteps = prcn - 1
    assert nsteps >= 1, "prcn must be >= 2"
    assert net_Iext.shape == (R, C) and J.shape == (N, N)

    if np.any(net_r0 != 0):
        return _numpy_fallback(net_Iext, J, net_r0, prcn)

    nc = _nc_cache if _nc_cache is not None else _get_compiled(nsteps)

    ident = np.eye(128, dtype=np.float32)
    iflat = net_Iext.ravel()
    in_maps = []
    for m in range(NCORES):
        # rotate window blocks so loop position p holds global window
        # (m + p) % NW  (position 0 = this core's own window)
        rows = J[m * LOCAL:(m + 1) * LOCAL, :]
        jm = np.empty_like(rows)
        for p in range(NW):
            w = (m + p) % NW
            jm[:, p * L:(p + 1) * L] = rows[:, w * L:(w + 1) * L]
        in_maps.append({
            "jm": jm,
            "iext": np.ascontiguousarray(
                iflat[m * LOCAL:(m + 1) * LOCAL].reshape(NT, 128).T),
            "ident": ident,
        })

    res = _run(nc, in_maps, trace=_trace)

    u_full = np.empty(N, dtype=np.float32)
    b_full = np.empty(N, dtype=np.float32)
    s_parts = []
    for m in range(NCORES):
        r_m = res.results[m]
        u_full[m * LOCAL:(m + 1) * LOCAL] = r_m["u_out"].T.ravel()
        b_full[m * LOCAL:(m + 1) * LOCAL] = r_m["b_out"].T.ravel()
        s_parts.append(np.float32(r_m["s_out"][0, 0]))
    s = np.float32(0.0)
    for p in s_parts:
        s = np.float32(s + p)
    r_full = ((b_full * (P04 / s)) / P04).astype(np.float32)

    out = (u_full.reshape(R, C), s, r_full.reshape(R, C))
    if _trace:
        out = (out, res)
    return out
